# revision 31
# baseline (speedup 1.0000x reference)
"""Trainium2 Bass kernel for nn_LocalEnhancementModule (8-core SPMD, data-parallel over batch).

Per-sample computation (B=16, P=256 patches, D=4096, E=512):
    p      = patchify(x)                       [P, D]
    theta  = p @ theta_w + theta_b             [P, E]
    f      = p @ f_w + f_b                     [P, E]
    wgt    = softmax(theta @ f.T, axis=-1)     [P, P]
    g      = p @ g_w + g_b                     [P, D]
    out    = unpatchify(wgt[:,None,:] * g.reshape(P,C,P)) * scale + x

Sharding: 2 samples per core. Host pre-patchifies x. theta/f projections run
in fp16 (softmax argmax is sensitive to score noise); the dominant g
projection (80% of FLOPs) runs in fp8-e4m3 with MatmulPerfMode.DoubleRow
(2 contraction k-tiles per instruction). g_w is pre-scaled by 64 on the host
so its sigma~0.02 values clear e4m3's 2^-6 subnormal floor; the 1/64 is
folded into the softmax weights. PSUM accumulates fp32; softmax runs in
fp32; the residual patch tensor streams as fp16 and the output is stored
fp16 (upcast on host).

Schedule notes: each dma_start costs ~0.6us of issuing-queue time, so
transfers are batched (k-pair weight/patch tiles, quad-packed g_w slabs,
round-paired residual tiles) to keep both HWDGE queues far from saturation.
The fp8 copy of p is derived on-device by the otherwise-idle vector engine.
theta/f run k-outer into 8 PSUM banks; scores+softmax; then the g projection
rotates 8 PSUM accumulators over single-d rounds so two rounds are always in
flight. The last round's g_w slab is prefetched on the scalar ring and runs
k-inner per group so the final gating overlaps the remaining matmuls.
"""

import sys
import numpy as np
import ml_dtypes

try:
    import concourse.bacc as bacc
except ImportError:  # pragma: no cover
    for _p in ("/opt/trn_rl_repo", "/root/.axon_site/_ro/trn_rl_repo"):
        if _p not in sys.path:
            sys.path.append(_p)
    import concourse.bacc as bacc
import concourse.mybir as mybir
import concourse.tile as tile
from concourse.bass_utils import run_bass_kernel_spmd

NCORES = 8
B, C, H, W = 16, 16, 256, 256
NPS, PH, PW = 16, 16, 16
P = NPS * NPS            # 256 patches
D = C * PH * PW          # 4096
E = 512
SPC = B // NCORES        # 2 samples per core
PP = SPC * P             # 512 patch rows per core
KT = D // 128            # 32 contraction tiles
KP = KT // 2             # 16 DoubleRow contraction pairs
ET = E // 128            # 4 embedding chunks
DCH = D // 512           # 8 column chunks for g
GRP = [(s, pc) for s in range(SPC) for pc in range(2)]
GSCL = 64.0              # fp8 pre-scale on g_w (values ~N(0,0.02) vs e4m3
                         # min-normal 2^-6); compensated in softmax weights

F32 = mybir.dt.float32
F16 = mybir.dt.float16
F8 = mybir.dt.float8e4
DR = mybir.MatmulPerfMode.DoubleRow
NP_F8 = ml_dtypes.float8_e4m3

_built = {}
LAST_RESULTS = None  # stashed BassKernelResults for test harness introspection


def _build(with_tb, with_fb, with_gb):
    key = (with_tb, with_fb, with_gb)
    if key in _built:
        return _built[key]

    nc = bacc.Bacc("TRN2", num_devices=NCORES, debug=False)
    pt16_d = nc.dram_tensor("pt16", [D, PP], F16, kind="ExternalInput").ap()
    pnat_d = nc.dram_tensor("pnat", [PP, D], F16, kind="ExternalInput").ap()
    tw_d = nc.dram_tensor("tw", [D, E], F16, kind="ExternalInput").ap()
    fw_d = nc.dram_tensor("fw", [D, E], F16, kind="ExternalInput").ap()
    gw_d = nc.dram_tensor("gw", [KP * 128, DCH * 1024], F8, kind="ExternalInput").ap()
    tb_d = nc.dram_tensor("tb", [E, 1], F32, kind="ExternalInput").ap() if with_tb else None
    fb_d = nc.dram_tensor("fb", [E, 1], F32, kind="ExternalInput").ap() if with_fb else None
    gb_d = nc.dram_tensor("gb", [1, D], F32, kind="ExternalInput").ap() if with_gb else None
    out_d = nc.dram_tensor("out", [PP, D], F16, kind="ExternalOutput").ap()

    def pair(dram, k0, cols):
        # [2*128, cols] rows (k0, k0+1) viewed as [128, 2, cols]
        return dram[k0 * 128:(k0 + 2) * 128, :].rearrange("(j p) v -> p j v", j=2)

    with tile.TileContext(nc) as tc:
        with tc.tile_pool(name="persist", bufs=1) as pp_, \
             tc.tile_pool(name="ptstream", bufs=10) as ptp, \
             tc.tile_pool(name="wstream", bufs=16) as wp, \
             tc.tile_pool(name="gstream", bufs=4) as gp, \
             tc.tile_pool(name="pnstream", bufs=16) as pnp, \
             tc.tile_pool(name="enh", bufs=6) as ep, \
             tc.tile_pool(name="sm", bufs=2) as smp:

            bias_sb = {0: [], 1: []}
            for wi, bd in ((0, tb_d), (1, fb_d)):
                if bd is None:
                    continue
                for e in range(ET):
                    bt = pp_.tile([128, 1], F32, name=f"bias_{wi}_{e}", tag=f"bias_{wi}_{e}")
                    nc.scalar.dma_start(out=bt[:, :], in_=bd[e * 128:(e + 1) * 128, :])
                    bias_sb[wi].append(bt)
            gb_sb = None
            if gb_d is not None:
                gb_sb = pp_.tile([1, D], F32, name="gb_sb", tag="gb_sb")
                nc.scalar.dma_start(out=gb_sb[:, :], in_=gb_d[:, :])

            # fp8 stationary patch tiles for the g projection (DoubleRow
            # layout: [ki, ko, p], k = 2*kp + ko), derived on-device by the
            # vector engine from the pt16 stream.
            pt8 = [pp_.tile([128, 2, PP], F8, name=f"pt8_{kp}", tag=f"pt8_{kp}")
                   for kp in range(KP)]

            # ---- theta / f projections, k-outer into 8 PSUM banks ----
            with tc.tile_pool(name="psA", bufs=1, space="PSUM") as psA:
                ps_attn = {}
                for wi in (0, 1):
                    for e in range(ET):
                        ps_attn[(wi, e)] = psA.tile([128, PP], F32,
                                                    name=f"ps_attn_{wi}_{e}",
                                                    tag=f"attn_{wi}_{e}")

                def tf_mms(wslices, t16, k):
                    for wi in (0, 1):
                        for e in range(ET):
                            nc.tensor.matmul(ps_attn[(wi, e)][:, :],
                                             wslices[wi][:, e * 128:(e + 1) * 128],
                                             t16[:, :],
                                             start=(k == 0), stop=(k == KT - 1))

                # k = 0, 1 as single tiles so the first matmul's inputs are
                # small and land quickly after DMA-ring spin-up
                # column-chunked first-tile DMAs: descriptors round-robin
                # across DMA engines per dma_start, so 4 chunks transfer ~4x
                # faster than one big DMA right after ring spin-up
                def chunked_dma(ring, dst, src, n=4):
                    w = dst.shape[-1] // n
                    for c in range(n):
                        ring.dma_start(out=dst[:, c * w:(c + 1) * w],
                                       in_=src[:, c * w:(c + 1) * w])

                t16s, wts = [], []
                for k in (0, 1, 2, 3):
                    t16 = ptp.tile([128, PP], F16, name=f"pt16_{k}", tag="pt16s")
                    chunked_dma(nc.scalar if k % 2 == 0 else nc.sync,
                                t16, pt16_d[k * 128:(k + 1) * 128, :],
                                n=4 if k < 2 else 2)
                    wpair = []
                    for wi, wd in ((0, tw_d), (1, fw_d)):
                        wt = wp.tile([128, E], F16, name=f"wt_{wi}_{k}", tag="ws")
                        chunked_dma(nc.sync if wi == 0 else nc.scalar,
                                    wt, wd[k * 128:(k + 1) * 128, :],
                                    n=4 if k < 2 else 2)
                        wpair.append(wt)
                    tf_mms(wpair, t16, k)
                    t16s.append(t16)
                for kq in (0, 1):
                    nc.vector.tensor_copy(pt8[kq][:, 0, :], t16s[2 * kq][:, :])
                    nc.vector.tensor_copy(pt8[kq][:, 1, :], t16s[2 * kq + 1][:, :])

                # k >= 4: paired tiles (one DMA per k-pair per tensor)
                for kp in range(2, KP):
                    k0 = 2 * kp
                    t16p = ptp.tile([128, 2, PP], F16, name=f"pt16p_{kp}", tag="pt16p")
                    ring = nc.scalar if kp % 2 == 0 else nc.sync
                    ring.dma_start(out=t16p[:, :, :], in_=pair(pt16_d, k0, PP))
                    nc.vector.tensor_copy(pt8[kp][:, :, :], t16p[:, :, :])
                    w2 = {}
                    for wi, wd in ((0, tw_d), (1, fw_d)):
                        wt2 = wp.tile([128, 2, E], F16, name=f"wt2_{wi}_{kp}", tag="wp")
                        (nc.sync if wi == 0 else nc.scalar).dma_start(
                            out=wt2[:, :, :], in_=pair(wd, k0, E))
                        w2[wi] = wt2
                    for j in (0, 1):
                        tf_mms({0: w2[0][:, j, :], 1: w2[1][:, j, :]},
                               t16p[:, j, :], k0 + j)

                # PSUM -> SBUF fp16, all on the vector engine (the scalar
                # queue may be busy issuing DMAs; these are on the critical
                # path to the scores matmuls). e-major so scores e=0 starts
                # after two copies.
                proj_sb = {}
                for e in range(ET):
                    for wi in (0, 1):
                        sb = pp_.tile([128, PP], F16, name=f"proj_{wi}_{e}",
                                      tag=f"proj_{wi}_{e}")
                        if bias_sb[wi]:
                            nc.scalar.activation(sb[:, :], ps_attn[(wi, e)][:, :],
                                                 mybir.ActivationFunctionType.Identity,
                                                 bias=bias_sb[wi][e][:, :], scale=1.0)
                        elif wi == 0:
                            nc.vector.tensor_copy(sb[:, :], ps_attn[(wi, e)][:, :])
                        else:
                            nc.scalar.copy(sb[:, :], ps_attn[(wi, e)][:, :])
                        proj_sb[(wi, e)] = sb

            # ---- scores + softmax per (sample, p-chunk) ----
            # The final normalization folds in 1/GSCL to undo the fp8 g_w
            # pre-scale (wgt only ever multiplies g).
            wgt = {}
            with tc.tile_pool(name="psB", bufs=1, space="PSUM") as psB:
                for (s, pc) in GRP:
                    sps = psB.tile([128, P], F32, name=f"ps_sc_{s}_{pc}", tag="sc", bufs=4)
                    col = s * P + pc * 128
                    for e in range(ET):
                        nc.tensor.matmul(sps[:, :],
                                         proj_sb[(0, e)][:, col:col + 128],
                                         proj_sb[(1, e)][:, s * P:(s + 1) * P],
                                         start=(e == 0), stop=(e == ET - 1))
                    mx = smp.tile([128, 1], F32, name=f"mx_{s}_{pc}", tag="mx")
                    nc.vector.tensor_reduce(out=mx[:, :], in_=sps[:, :],
                                            axis=mybir.AxisListType.X, op=mybir.AluOpType.max)
                    ngm = smp.tile([128, 1], F32, name=f"ngm_{s}_{pc}", tag="ngm")
                    nc.vector.tensor_scalar_mul(ngm[:, :], mx[:, :], -1.0)
                    ex = smp.tile([128, P], F32, name=f"ex_{s}_{pc}", tag="ex")
                    ssum = smp.tile([128, 1], F32, name=f"ssum_{s}_{pc}", tag="ssum")
                    nc.scalar.activation(ex[:, :], sps[:, :], mybir.ActivationFunctionType.Exp,
                                         bias=ngm[:, :], scale=1.0, accum_out=ssum[:, :])
                    rec = smp.tile([128, 1], F32, name=f"rec_{s}_{pc}", tag="rec")
                    nc.vector.reciprocal(rec[:, :], ssum[:, :])
                    rec2 = smp.tile([128, 1], F32, name=f"rec2_{s}_{pc}", tag="rec2")
                    nc.vector.tensor_scalar_mul(rec2[:, :], rec[:, :], 1.0 / GSCL)
                    wt_ = pp_.tile([128, P], F32, name=f"wgt_{s}_{pc}", tag=f"wgt_{s}_{pc}")
                    nc.vector.tensor_scalar_mul(wt_[:, :], ex[:, :], rec2[:, :])
                    wgt[(s, pc)] = wt_

            # ---- g projection + gating + residual: fp8 DoubleRow ----
            LAST = DCH - 1
            gs_last = []
            with tc.tile_pool(name="psC", bufs=1, space="PSUM") as psC:

                pn_tiles = {}

                def prefetch_pn(dp):
                    # residual tiles for rounds (2*dp, 2*dp+1) in one DMA each
                    for (s, pc) in GRP:
                        row = s * P + pc * 128
                        pn2 = pnp.tile([128, 1024], F16, name=f"pn_{dp}_{s}_{pc}", tag="pn")
                        nc.scalar.dma_start(
                            out=pn2[:, :],
                            in_=pnat_d[row:row + 128, dp * 1024:(dp + 1) * 1024])
                        pn_tiles[(dp, s, pc)] = pn2

                def gate_group(d, dcol, s, pc, g_ps, split):
                    row = s * P + pc * 128
                    if gb_sb is not None:
                        nc.vector.tensor_add(
                            g_ps[:, :], g_ps[:, :],
                            gb_sb[0:1, dcol:dcol + 512].partition_broadcast(128))
                    en = ep.tile([128, 512], F16, name=f"en_{d}_{s}_{pc}", tag="en")
                    tmp = ep.tile([128, 512], F32, name=f"tmp_{d}_{s}_{pc}", tag="tmp")
                    pn2 = pn_tiles[(d // 2, s, pc)]
                    po = (d % 2) * 512
                    for (lo, hi) in ((0, 256), (256, 512)):
                        nc.vector.tensor_mul(tmp[:, lo:hi], g_ps[:, lo:hi],
                                             wgt[(s, pc)][:, :])
                    nc.vector.tensor_add(en[:, :], tmp[:, :], pn2[:, po:po + 512])
                    ring = nc.sync if split and (s + pc) % 2 == 0 else nc.scalar
                    ring.dma_start(out=out_d[row:row + 128, dcol:dcol + 512],
                                   in_=en[:, :])
                    if d % 2 == 1:
                        del pn_tiles[(d // 2, s, pc)]

                # residuals for rounds 0-3 upfront (rings have slack in late
                # theta/f), the rest early in the g phase so no residual
                # loads remain near the end-of-kernel drain
                prefetch_pn(0)
                prefetch_pn(1)
                for d in range(LAST):
                    dcol = d * 512
                    if d in (0, 2):
                        prefetch_pn(d // 2 + 2)
                    gps = {}
                    for (s, pc) in GRP:
                        gps[(s, pc)] = psC.tile([128, 512], F32,
                                                name=f"ps_g_{d}_{s}_{pc}", tag="g", bufs=6)
                    quads = []
                    for q in range(4):
                        gt4 = gp.tile([128, 4, 2, 512], F8, name=f"gt_{d}_{q}", tag="gt")
                        nc.sync.dma_start(
                            out=gt4[:, :, :, :],
                            in_=gw_d[q * 512:(q + 1) * 512,
                                     d * 1024:(d + 1) * 1024].rearrange(
                                         "(j p) v -> p j v", j=4))
                        quads.append(gt4)
                    for kp in range(KP):
                        gt = quads[kp // 4][:, kp % 4, :, :]
                        for (s, pc) in GRP:
                            col = s * P + pc * 128
                            nc.tensor.matmul(gps[(s, pc)][:, :],
                                             pt8[kp][:, :, col:col + 128],
                                             gt,
                                             start=(kp == 0), stop=(kp == KP - 1),
                                             perf_mode=DR)
                    for (s, pc) in GRP:
                        gate_group(d, dcol, s, pc, gps[(s, pc)], split=False)
                    if d == DCH - 3:
                        # prefetch the last round's g_w slab on the scalar ring
                        for q in range(4):
                            gl4 = pp_.tile([128, 4, 2, 512], F8, name=f"gs_last_{q}",
                                           tag=f"gs_last_{q}")
                            nc.scalar.dma_start(
                                out=gl4[:, :, :, :],
                                in_=gw_d[q * 512:(q + 1) * 512,
                                         LAST * 1024:(LAST + 1) * 1024].rearrange(
                                             "(j p) v -> p j v", j=4))
                            gs_last.append(gl4)

                dcol = LAST * 512
                for (s, pc) in GRP:
                    col = s * P + pc * 128
                    g_ps = psC.tile([128, 512], F32,
                                    name=f"ps_g_{LAST}_{s}_{pc}", tag="g", bufs=6)
                    for kp in range(KP):
                        nc.tensor.matmul(g_ps[:, :], pt8[kp][:, :, col:col + 128],
                                         gs_last[kp // 4][:, kp % 4, :, :],
                                         start=(kp == 0), stop=(kp == KP - 1),
                                         perf_mode=DR)
                    gate_group(LAST, dcol, s, pc, g_ps, split=True)

    nc.compile()
    _built[key] = nc
    return nc


def kernel(**inputs):
    global LAST_RESULTS
    x = np.ascontiguousarray(inputs["x"], dtype=np.float32)
    tw = np.asarray(inputs["theta_w"], dtype=np.float32)
    fw = np.asarray(inputs["f_w"], dtype=np.float32)
    gw = np.asarray(inputs["g_w"], dtype=np.float32)
    tb = np.asarray(inputs["theta_b"], dtype=np.float32)
    fb = np.asarray(inputs["f_b"], dtype=np.float32)
    gb = np.asarray(inputs["g_b"], dtype=np.float32)
    scale = float(np.asarray(inputs["scale"], dtype=np.float32).reshape(-1)[0])

    with_tb = bool(np.any(tb))
    with_fb = bool(np.any(fb))
    with_gb = bool(np.any(gb))
    nc = _build(with_tb, with_fb, with_gb)

    # patchify: [B,C,H,W] -> [B,P,D] with D ordered (c, u, v)
    p = x.reshape(B, C, NPS, PH, NPS, PW).transpose(0, 2, 4, 1, 3, 5).reshape(B, P, D)
    tw16 = np.ascontiguousarray(tw).astype(np.float16)
    fw16 = np.ascontiguousarray(fw).astype(np.float16)
    # g_w in fp8 e4m3, pre-scaled by GSCL (and the module's output scale);
    # packed for DoubleRow: row (kp*128+ki), free (d-slab, ko, 512)
    g8 = (gw * (GSCL * scale)).astype(NP_F8)
    gw8 = np.ascontiguousarray(
        g8.reshape(KP, 2, 128, DCH, 512).transpose(0, 2, 3, 1, 4).reshape(KP * 128, DCH * 1024))
    in_maps = []
    for ci in range(NCORES):
        p2 = p[ci * SPC:(ci + 1) * SPC]                      # [SPC, P, D]
        pnat = np.ascontiguousarray(p2.reshape(PP, D)).astype(np.float16)
        pT16 = np.ascontiguousarray(p2.transpose(2, 0, 1).reshape(D, PP)).astype(np.float16)
        m = {"pt16": pT16, "pnat": pnat, "tw": tw16, "fw": fw16, "gw": gw8}
        if with_tb:
            m["tb"] = np.ascontiguousarray(tb.reshape(E, 1))
        if with_fb:
            m["fb"] = np.ascontiguousarray(fb.reshape(E, 1))
        if with_gb:
            m["gb"] = np.ascontiguousarray((scale * gb).reshape(1, D))
        in_maps.append(m)

    res = run_bass_kernel_spmd(nc, in_maps, core_ids=list(range(NCORES)))
    LAST_RESULTS = res
    o = np.concatenate([np.asarray(res.results[ci]["out"], dtype=np.float32)
                        .reshape(SPC, P, D) for ci in range(NCORES)], axis=0)
    img = (o.reshape(B, NPS, NPS, C, PH, PW)
            .transpose(0, 3, 1, 4, 2, 5)
            .reshape(B, C, H, W))
    return np.ascontiguousarray(img, dtype=np.float32)


# revision 34
# speedup vs baseline: 1.0476x; 1.0476x over previous
"""Trainium2 Bass kernel for nn_LocalEnhancementModule (8-core SPMD, data-parallel over batch).

Per-sample computation (B=16, P=256 patches, D=4096, E=512):
    p      = patchify(x)                       [P, D]
    theta  = p @ theta_w + theta_b             [P, E]
    f      = p @ f_w + f_b                     [P, E]
    wgt    = softmax(theta @ f.T, axis=-1)     [P, P]
    g      = p @ g_w + g_b                     [P, D]
    out    = unpatchify(wgt[:,None,:] * g.reshape(P,C,P)) * scale + x

Sharding: 2 samples per core. Host pre-patchifies x. theta/f projections run
in fp16 (softmax argmax is sensitive to score noise); the dominant g
projection (80% of FLOPs) runs in fp8-e4m3 with MatmulPerfMode.DoubleRow
(2 contraction k-tiles per instruction). g_w is pre-scaled by 64 on the host
so its sigma~0.02 values clear e4m3's 2^-6 subnormal floor; the 1/64 is
folded into the softmax weights. PSUM accumulates fp32; softmax runs in
fp32; the residual patch tensor streams as fp16 and the output is stored
fp16 (upcast on host).

Schedule notes: each dma_start costs ~0.6us of issuing-queue time, so
transfers are batched (k-pair weight/patch tiles, quad-packed g_w slabs,
round-paired residual tiles) to keep both HWDGE queues far from saturation.
The fp8 copy of p is derived on-device by the otherwise-idle vector engine.
theta/f run k-outer into 8 PSUM banks; scores+softmax; then the g projection
rotates 8 PSUM accumulators over single-d rounds so two rounds are always in
flight. The last round's g_w slab is prefetched on the scalar ring and runs
k-inner per group so the final gating overlaps the remaining matmuls.
"""

import sys
import numpy as np
import ml_dtypes

try:
    import concourse.bacc as bacc
except ImportError:  # pragma: no cover
    for _p in ("/opt/trn_rl_repo", "/root/.axon_site/_ro/trn_rl_repo"):
        if _p not in sys.path:
            sys.path.append(_p)
    import concourse.bacc as bacc
import concourse.mybir as mybir
import concourse.tile as tile
from concourse.bass_utils import run_bass_kernel_spmd

NCORES = 8
B, C, H, W = 16, 16, 256, 256
NPS, PH, PW = 16, 16, 16
P = NPS * NPS            # 256 patches
D = C * PH * PW          # 4096
E = 512
SPC = B // NCORES        # 2 samples per core
PP = SPC * P             # 512 patch rows per core
KT = D // 128            # 32 contraction tiles
KP = KT // 2             # 16 DoubleRow contraction pairs
ET = E // 128            # 4 embedding chunks
DCH = D // 512           # 8 column chunks for g
GRP = [(s, pc) for s in range(SPC) for pc in range(2)]
GSCL = 64.0              # fp8 pre-scale on g_w (values ~N(0,0.02) vs e4m3
                         # min-normal 2^-6); compensated in softmax weights

F32 = mybir.dt.float32
F16 = mybir.dt.float16
F8 = mybir.dt.float8e4
DR = mybir.MatmulPerfMode.DoubleRow
NP_F8 = ml_dtypes.float8_e4m3

_built = {}
LAST_RESULTS = None  # stashed BassKernelResults for test harness introspection


def _build(with_tb, with_fb, with_gb):
    key = (with_tb, with_fb, with_gb)
    if key in _built:
        return _built[key]

    nc = bacc.Bacc("TRN2", num_devices=NCORES, debug=False)
    pt16_d = nc.dram_tensor("pt16", [D, PP], F16, kind="ExternalInput").ap()
    pnat_d = nc.dram_tensor("pnat", [PP, D], F16, kind="ExternalInput").ap()
    tw_d = nc.dram_tensor("tw", [D, E], F16, kind="ExternalInput").ap()
    fw_d = nc.dram_tensor("fw", [D, E], F16, kind="ExternalInput").ap()
    gw_d = nc.dram_tensor("gw", [KP * 128, DCH * 1024], F8, kind="ExternalInput").ap()
    tb_d = nc.dram_tensor("tb", [E, 1], F32, kind="ExternalInput").ap() if with_tb else None
    fb_d = nc.dram_tensor("fb", [E, 1], F32, kind="ExternalInput").ap() if with_fb else None
    gb_d = nc.dram_tensor("gb", [1, D], F32, kind="ExternalInput").ap() if with_gb else None
    out_d = nc.dram_tensor("out", [PP, D], F16, kind="ExternalOutput").ap()

    def pair(dram, k0, cols):
        # [2*128, cols] rows (k0, k0+1) viewed as [128, 2, cols]
        return dram[k0 * 128:(k0 + 2) * 128, :].rearrange("(j p) v -> p j v", j=2)

    with tile.TileContext(nc) as tc:
        with tc.tile_pool(name="persist", bufs=1) as pp_, \
             tc.tile_pool(name="ptstream", bufs=10) as ptp, \
             tc.tile_pool(name="wstream", bufs=16) as wp, \
             tc.tile_pool(name="gstream", bufs=4) as gp, \
             tc.tile_pool(name="pnstream", bufs=16) as pnp, \
             tc.tile_pool(name="enh", bufs=6) as ep, \
             tc.tile_pool(name="sm", bufs=2) as smp:

            bias_sb = {0: [], 1: []}
            for wi, bd in ((0, tb_d), (1, fb_d)):
                if bd is None:
                    continue
                for e in range(ET):
                    bt = pp_.tile([128, 1], F32, name=f"bias_{wi}_{e}", tag=f"bias_{wi}_{e}")
                    nc.scalar.dma_start(out=bt[:, :], in_=bd[e * 128:(e + 1) * 128, :])
                    bias_sb[wi].append(bt)
            gb_sb = None
            if gb_d is not None:
                gb_sb = pp_.tile([1, D], F32, name="gb_sb", tag="gb_sb")
                nc.scalar.dma_start(out=gb_sb[:, :], in_=gb_d[:, :])

            # fp8 stationary patch tiles for the g projection (DoubleRow
            # layout: [ki, ko, p], k = 2*kp + ko), derived on-device by the
            # vector engine from the pt16 stream.
            pt8 = [pp_.tile([128, 2, PP], F8, name=f"pt8_{kp}", tag=f"pt8_{kp}")
                   for kp in range(KP)]

            # ---- theta / f projections, k-outer into 8 PSUM banks ----
            with tc.tile_pool(name="psA", bufs=1, space="PSUM") as psA:
                ps_attn = {}
                for wi in (0, 1):
                    for e in range(ET):
                        ps_attn[(wi, e)] = psA.tile([128, PP], F32,
                                                    name=f"ps_attn_{wi}_{e}",
                                                    tag=f"attn_{wi}_{e}")

                def tf_mms(wslices, t16, k):
                    for wi in (0, 1):
                        for e in range(ET):
                            nc.tensor.matmul(ps_attn[(wi, e)][:, :],
                                             wslices[wi][:, e * 128:(e + 1) * 128],
                                             t16[:, :],
                                             start=(k == 0), stop=(k == KT - 1))

                # k = 0, 1 as single tiles so the first matmul's inputs are
                # small and land quickly after DMA-ring spin-up
                # column-chunked first-tile DMAs: descriptors round-robin
                # across DMA engines per dma_start, so 4 chunks transfer ~4x
                # faster than one big DMA right after ring spin-up
                def chunked_dma(ring, dst, src, n=4):
                    w = dst.shape[-1] // n
                    for c in range(n):
                        ring.dma_start(out=dst[:, c * w:(c + 1) * w],
                                       in_=src[:, c * w:(c + 1) * w])

                t16s, wts = [], []
                for k in (0, 1, 2, 3):
                    t16 = ptp.tile([128, PP], F16, name=f"pt16_{k}", tag="pt16s")
                    chunked_dma(nc.scalar if k % 2 == 0 else nc.sync,
                                t16, pt16_d[k * 128:(k + 1) * 128, :],
                                n=4 if k < 2 else 2)
                    wpair = []
                    for wi, wd in ((0, tw_d), (1, fw_d)):
                        wt = wp.tile([128, E], F16, name=f"wt_{wi}_{k}", tag="ws")
                        chunked_dma(nc.sync if wi == 0 else nc.scalar,
                                    wt, wd[k * 128:(k + 1) * 128, :],
                                    n=4 if k < 2 else 2)
                        wpair.append(wt)
                    tf_mms(wpair, t16, k)
                    t16s.append(t16)
                for kq in (0, 1):
                    nc.vector.tensor_copy(pt8[kq][:, 0, :], t16s[2 * kq][:, :])
                    nc.vector.tensor_copy(pt8[kq][:, 1, :], t16s[2 * kq + 1][:, :])

                # k >= 4: paired tiles (one DMA per k-pair per tensor)
                for kp in range(2, KP):
                    k0 = 2 * kp
                    t16p = ptp.tile([128, 2, PP], F16, name=f"pt16p_{kp}", tag="pt16p")
                    ring = nc.scalar if kp % 2 == 0 else nc.sync
                    ring.dma_start(out=t16p[:, :, :], in_=pair(pt16_d, k0, PP))
                    nc.vector.tensor_copy(pt8[kp][:, :, :], t16p[:, :, :])
                    w2 = {}
                    for wi, wd in ((0, tw_d), (1, fw_d)):
                        wt2 = wp.tile([128, 2, E], F16, name=f"wt2_{wi}_{kp}", tag="wp")
                        (nc.sync if wi == 0 else nc.scalar).dma_start(
                            out=wt2[:, :, :], in_=pair(wd, k0, E))
                        w2[wi] = wt2
                    for j in (0, 1):
                        tf_mms({0: w2[0][:, j, :], 1: w2[1][:, j, :]},
                               t16p[:, j, :], k0 + j)

                # PSUM -> SBUF fp16, all on the vector engine (the scalar
                # queue may be busy issuing DMAs; these are on the critical
                # path to the scores matmuls). e-major so scores e=0 starts
                # after two copies.
                proj_sb = {}
                for e in range(ET):
                    for wi in (0, 1):
                        sb = pp_.tile([128, PP], F16, name=f"proj_{wi}_{e}",
                                      tag=f"proj_{wi}_{e}")
                        if bias_sb[wi]:
                            nc.scalar.activation(sb[:, :], ps_attn[(wi, e)][:, :],
                                                 mybir.ActivationFunctionType.Identity,
                                                 bias=bias_sb[wi][e][:, :], scale=1.0)
                        elif wi == 0:
                            nc.vector.tensor_copy(sb[:, :], ps_attn[(wi, e)][:, :])
                        else:
                            nc.scalar.copy(sb[:, :], ps_attn[(wi, e)][:, :])
                        proj_sb[(wi, e)] = sb

            # ---- scores + softmax (emitted between g round 0's matmuls and
            # its gates, so the PE flows straight from theta/f into the g
            # projection while softmax runs on vector/scalar) ----
            # The final normalization folds in 1/GSCL to undo the fp8 g_w
            # pre-scale (wgt only ever multiplies g).
            wgt = {}

            def emit_scores(psB):
                for (s, pc) in GRP:
                    sps = psB.tile([128, P], F32, name=f"ps_sc_{s}_{pc}", tag="sc", bufs=2)
                    col = s * P + pc * 128
                    for e in range(ET):
                        nc.tensor.matmul(sps[:, :],
                                         proj_sb[(0, e)][:, col:col + 128],
                                         proj_sb[(1, e)][:, s * P:(s + 1) * P],
                                         start=(e == 0), stop=(e == ET - 1))
                    mx = smp.tile([128, 1], F32, name=f"mx_{s}_{pc}", tag="mx")
                    nc.vector.tensor_reduce(out=mx[:, :], in_=sps[:, :],
                                            axis=mybir.AxisListType.X, op=mybir.AluOpType.max)
                    ngm = smp.tile([128, 1], F32, name=f"ngm_{s}_{pc}", tag="ngm")
                    nc.vector.tensor_scalar_mul(ngm[:, :], mx[:, :], -1.0)
                    ex = smp.tile([128, P], F32, name=f"ex_{s}_{pc}", tag="ex")
                    ssum = smp.tile([128, 1], F32, name=f"ssum_{s}_{pc}", tag="ssum")
                    nc.scalar.activation(ex[:, :], sps[:, :], mybir.ActivationFunctionType.Exp,
                                         bias=ngm[:, :], scale=1.0, accum_out=ssum[:, :])
                    rec = smp.tile([128, 1], F32, name=f"rec_{s}_{pc}", tag="rec")
                    nc.vector.reciprocal(rec[:, :], ssum[:, :])
                    rec2 = smp.tile([128, 1], F32, name=f"rec2_{s}_{pc}", tag="rec2")
                    nc.vector.tensor_scalar_mul(rec2[:, :], rec[:, :], 1.0 / GSCL)
                    wt_ = pp_.tile([128, P], F32, name=f"wgt_{s}_{pc}", tag=f"wgt_{s}_{pc}")
                    nc.vector.tensor_scalar_mul(wt_[:, :], ex[:, :], rec2[:, :])
                    wgt[(s, pc)] = wt_

            # ---- g projection + gating + residual: fp8 DoubleRow ----
            LAST = DCH - 1
            gs_last = []
            with tc.tile_pool(name="psB", bufs=1, space="PSUM") as psB, \
                 tc.tile_pool(name="psC", bufs=1, space="PSUM") as psC:

                pn_tiles = {}

                def prefetch_pn(dp):
                    # residual tiles for rounds (2*dp, 2*dp+1) in one DMA each
                    for (s, pc) in GRP:
                        row = s * P + pc * 128
                        pn2 = pnp.tile([128, 1024], F16, name=f"pn_{dp}_{s}_{pc}", tag="pn")
                        nc.scalar.dma_start(
                            out=pn2[:, :],
                            in_=pnat_d[row:row + 128, dp * 1024:(dp + 1) * 1024])
                        pn_tiles[(dp, s, pc)] = pn2

                def gate_group(d, dcol, s, pc, g_ps, split):
                    row = s * P + pc * 128
                    if gb_sb is not None:
                        nc.vector.tensor_add(
                            g_ps[:, :], g_ps[:, :],
                            gb_sb[0:1, dcol:dcol + 512].partition_broadcast(128))
                    en = ep.tile([128, 512], F16, name=f"en_{d}_{s}_{pc}", tag="en")
                    tmp = ep.tile([128, 512], F32, name=f"tmp_{d}_{s}_{pc}", tag="tmp")
                    pn2 = pn_tiles[(d // 2, s, pc)]
                    po = (d % 2) * 512
                    for (lo, hi) in ((0, 256), (256, 512)):
                        nc.vector.tensor_mul(tmp[:, lo:hi], g_ps[:, lo:hi],
                                             wgt[(s, pc)][:, :])
                    nc.vector.tensor_add(en[:, :], tmp[:, :], pn2[:, po:po + 512])
                    ring = nc.sync if split and (s + pc) % 2 == 0 else nc.scalar
                    ring.dma_start(out=out_d[row:row + 128, dcol:dcol + 512],
                                   in_=en[:, :])
                    if d % 2 == 1:
                        del pn_tiles[(d // 2, s, pc)]

                # residuals for rounds 0-3 upfront (rings have slack in late
                # theta/f), the rest early in the g phase so no residual
                # loads remain near the end-of-kernel drain
                prefetch_pn(0)
                prefetch_pn(1)
                for d in range(LAST):
                    dcol = d * 512
                    if d in (0, 2):
                        prefetch_pn(d // 2 + 2)
                    gps = {}
                    for (s, pc) in GRP:
                        gps[(s, pc)] = psC.tile([128, 512], F32,
                                                name=f"ps_g_{d}_{s}_{pc}", tag="g", bufs=6)
                    quads = []
                    for q in range(4):
                        gt4 = gp.tile([128, 4, 2, 512], F8, name=f"gt_{d}_{q}", tag="gt")
                        nc.sync.dma_start(
                            out=gt4[:, :, :, :],
                            in_=gw_d[q * 512:(q + 1) * 512,
                                     d * 1024:(d + 1) * 1024].rearrange(
                                         "(j p) v -> p j v", j=4))
                        quads.append(gt4)
                    for kp in range(KP):
                        gt = quads[kp // 4][:, kp % 4, :, :]
                        for (s, pc) in GRP:
                            col = s * P + pc * 128
                            nc.tensor.matmul(gps[(s, pc)][:, :],
                                             pt8[kp][:, :, col:col + 128],
                                             gt,
                                             start=(kp == 0), stop=(kp == KP - 1),
                                             perf_mode=DR)
                    if d == 0:
                        emit_scores(psB)
                    for (s, pc) in GRP:
                        gate_group(d, dcol, s, pc, gps[(s, pc)], split=False)
                    if d == DCH - 3:
                        # prefetch the last round's g_w slab on the scalar ring
                        for q in range(4):
                            gl4 = pp_.tile([128, 4, 2, 512], F8, name=f"gs_last_{q}",
                                           tag=f"gs_last_{q}")
                            nc.scalar.dma_start(
                                out=gl4[:, :, :, :],
                                in_=gw_d[q * 512:(q + 1) * 512,
                                         LAST * 1024:(LAST + 1) * 1024].rearrange(
                                             "(j p) v -> p j v", j=4))
                            gs_last.append(gl4)

                dcol = LAST * 512
                for (s, pc) in GRP:
                    col = s * P + pc * 128
                    g_ps = psC.tile([128, 512], F32,
                                    name=f"ps_g_{LAST}_{s}_{pc}", tag="g", bufs=6)
                    for kp in range(KP):
                        nc.tensor.matmul(g_ps[:, :], pt8[kp][:, :, col:col + 128],
                                         gs_last[kp // 4][:, kp % 4, :, :],
                                         start=(kp == 0), stop=(kp == KP - 1),
                                         perf_mode=DR)
                    gate_group(LAST, dcol, s, pc, g_ps, split=True)

    nc.compile()
    _built[key] = nc
    return nc


def kernel(**inputs):
    global LAST_RESULTS
    x = np.ascontiguousarray(inputs["x"], dtype=np.float32)
    tw = np.asarray(inputs["theta_w"], dtype=np.float32)
    fw = np.asarray(inputs["f_w"], dtype=np.float32)
    gw = np.asarray(inputs["g_w"], dtype=np.float32)
    tb = np.asarray(inputs["theta_b"], dtype=np.float32)
    fb = np.asarray(inputs["f_b"], dtype=np.float32)
    gb = np.asarray(inputs["g_b"], dtype=np.float32)
    scale = float(np.asarray(inputs["scale"], dtype=np.float32).reshape(-1)[0])

    with_tb = bool(np.any(tb))
    with_fb = bool(np.any(fb))
    with_gb = bool(np.any(gb))
    nc = _build(with_tb, with_fb, with_gb)

    # patchify: [B,C,H,W] -> [B,P,D] with D ordered (c, u, v)
    p = x.reshape(B, C, NPS, PH, NPS, PW).transpose(0, 2, 4, 1, 3, 5).reshape(B, P, D)
    tw16 = np.ascontiguousarray(tw).astype(np.float16)
    fw16 = np.ascontiguousarray(fw).astype(np.float16)
    # g_w in fp8 e4m3, pre-scaled by GSCL (and the module's output scale);
    # packed for DoubleRow: row (kp*128+ki), free (d-slab, ko, 512)
    g8 = (gw * (GSCL * scale)).astype(NP_F8)
    gw8 = np.ascontiguousarray(
        g8.reshape(KP, 2, 128, DCH, 512).transpose(0, 2, 3, 1, 4).reshape(KP * 128, DCH * 1024))
    in_maps = []
    for ci in range(NCORES):
        p2 = p[ci * SPC:(ci + 1) * SPC]                      # [SPC, P, D]
        pnat = np.ascontiguousarray(p2.reshape(PP, D)).astype(np.float16)
        pT16 = np.ascontiguousarray(p2.transpose(2, 0, 1).reshape(D, PP)).astype(np.float16)
        m = {"pt16": pT16, "pnat": pnat, "tw": tw16, "fw": fw16, "gw": gw8}
        if with_tb:
            m["tb"] = np.ascontiguousarray(tb.reshape(E, 1))
        if with_fb:
            m["fb"] = np.ascontiguousarray(fb.reshape(E, 1))
        if with_gb:
            m["gb"] = np.ascontiguousarray((scale * gb).reshape(1, D))
        in_maps.append(m)

    res = run_bass_kernel_spmd(nc, in_maps, core_ids=list(range(NCORES)))
    LAST_RESULTS = res
    o = np.concatenate([np.asarray(res.results[ci]["out"], dtype=np.float32)
                        .reshape(SPC, P, D) for ci in range(NCORES)], axis=0)
    img = (o.reshape(B, NPS, NPS, C, PH, PW)
            .transpose(0, 3, 1, 4, 2, 5)
            .reshape(B, C, H, W))
    return np.ascontiguousarray(img, dtype=np.float32)
